# revision 1
# baseline (speedup 1.0000x reference)
"""Trainium2 Bass kernel for channel-attention (AttnBlock-style, contraction
over spatial axis) distributed over 8 NeuronCores.

Problem (hardcoded shapes):
  x: [16, 768, 64, 64] f32; wq/wk/wv/wo: [768, 768]; bq/bk/bv/bo: [768]
  q = wq@x+bq; k = ...; v = ...   (1x1 conv == per-pixel channel matmul)
  energy[b,h,i,j] = sum_n q[b,h,i,n] * k[b,h,j,n] * scale   (n = 4096 spatial)
  attn = softmax(energy, -1);  out[b,h,i,n] = sum_j attn[i,j] v[b,h,j,n]
  y = wo@out+bo

Sharding: pure data-parallel over batch (16 batches -> 2 per core), weights
replicated. No collectives needed.

Per-core dataflow (per batch, X = x[b] as [C=768, N=4096] in SBUF, bf16):
  Phase A: for each n-tile (128 spatial cols): QT/KT = X^T @ wqT/wkT + b
           produced in TRANSPOSED layout [n, c] (stationary = X tile), then
           12 per-head matmuls accumulate energy E[q,k] in PSUM over all 32
           n-tiles (contraction over spatial on the partition axis).
           Head pairs are packed into one [128, 64] PSUM region (head 2p ->
           partitions 0:64, head 2p+1 -> 64:128) via tile_position packing.
  Softmax: rowwise over k (free axis): -max, Exp(+accum row-sum), recip.
           A is left unnormalized; 1/sum is applied at O-eviction as a
           per-partition scale. A^T computed with PE transpose-mode.
  Phase V/O: V = wv@X + bv (normal [c, n] layout; bias per partition at
           PSUM eviction). Then per head pair O = A^T.T @ V with two
           concurrent 64x64-stationary matmuls (partitions 0:64 / 64:128
           of the array).
  Phase Y: y = woT.T @ O + bo' accumulated over 6 c-tiles, f32 out, DMA out.

Weights are pre-transposed/cast on host; SCALE is folded into wq/bq.
"""

import os
import sys
import numpy as np
import ml_dtypes

if "/opt/trn_rl_repo" not in sys.path:
    sys.path.insert(0, "/opt/trn_rl_repo")

B, C, HH, WW = 16, 768, 64, 64
NUM_HEADS = 12
HEAD_DIM = 64
SCALE = HEAD_DIM ** -0.5
N = HH * WW            # 4096 spatial positions
NCORES = 8
NB = B // NCORES       # batches per core = 2
P = 128
CT = C // P            # 6 channel tiles
NT = N // P            # 32 spatial tiles of 128
NCH = N // 512         # 8 spatial chunks of 512
NPAIR = NUM_HEADS // 2 # 6 head pairs

_CACHE = {}

# "bf16": all-bf16 compute (~1.2e-2 rel err). "splitqk": error-compensated
# Q/K projections via hi/lo bf16 split (X@W ~ Xh@Wh + Xh@Wl + Xl@Wh),
# ~5.3e-3 rel err at 3x Q/K-projection PE cost.
PRECISION = os.environ.get("KERNEL_PRECISION", "bf16")


def _build_nc(precision=None):
    import concourse.bass as bass
    import concourse.bacc as bacc
    import concourse.mybir as mybir
    from concourse.tile import TileContext
    from concourse.masks import make_identity
    from contextlib import ExitStack

    BF = mybir.dt.bfloat16
    F32 = mybir.dt.float32
    AX = mybir.AxisListType
    ACT = mybir.ActivationFunctionType

    if precision is None:
        precision = PRECISION
    split = precision == "splitqk"

    nc = bacc.Bacc("TRN2", target_bir_lowering=False, debug=False,
                   enable_asserts=False, num_devices=NCORES)

    x_p = nc.declare_dram_parameter("x", [NB, C, N], BF, isOutput=False)
    wqk_p = nc.declare_dram_parameter("wqk", [C, 2 * C], BF, isOutput=False)
    wvt_p = nc.declare_dram_parameter("wvt", [C, C], BF, isOutput=False)
    wot_p = nc.declare_dram_parameter("wot", [C, C], BF, isOutput=False)
    if split:
        xl_p = nc.declare_dram_parameter("xl", [NB, C, N], BF, isOutput=False)
        wqkl_p = nc.declare_dram_parameter("wqkl", [C, 2 * C], BF, isOutput=False)
    bqkb_p = nc.declare_dram_parameter("bqkb", [P, 2 * C], F32, isOutput=False)
    bvt_p = nc.declare_dram_parameter("bvt", [P, CT], F32, isOutput=False)
    bot_p = nc.declare_dram_parameter("bot", [P, CT], F32, isOutput=False)
    out_p = nc.declare_dram_parameter("out", [NB, C, N], F32, isOutput=True)

    with TileContext(nc) as tc, ExitStack() as ctx:
        const = ctx.enter_context(tc.tile_pool(name="const", bufs=1))
        x_pool = ctx.enter_context(tc.tile_pool(name="xp", bufs=CT))
        qk_pool = ctx.enter_context(tc.tile_pool(name="qkp", bufs=2 if split else 3))
        v_pool = ctx.enter_context(tc.tile_pool(name="vp", bufs=2))
        o_pool = ctx.enter_context(tc.tile_pool(name="op", bufs=CT))
        at_pool = ctx.enter_context(tc.tile_pool(name="atp", bufs=NPAIR))
        a_pool = ctx.enter_context(tc.tile_pool(name="ap", bufs=2))
        stat_pool = ctx.enter_context(tc.tile_pool(name="statp", bufs=4))
        rinv_pool = ctx.enter_context(tc.tile_pool(name="rinvp", bufs=2))
        y_pool = ctx.enter_context(tc.tile_pool(name="yp", bufs=1 if split else 2))
        e_pool = ctx.enter_context(tc.tile_pool(name="ep", bufs=2))
        if split:
            xl_pool = ctx.enter_context(tc.tile_pool(name="xlp", bufs=8))
        psA_pool = ctx.enter_context(tc.tile_pool(name="psA", bufs=3, space="PSUM"))
        psE_pool = ctx.enter_context(tc.tile_pool(name="psE", bufs=1, space="PSUM"))
        psat_pool = ctx.enter_context(tc.tile_pool(name="psat", bufs=1, space="PSUM"))
        ps512_pool = ctx.enter_context(tc.tile_pool(name="ps512", bufs=3, space="PSUM"))

        # --- constants: weights, biases, identity ---
        # Startup-critical DMAs only: the leading 512 cols of the merged
        # QK weight gate the first matmuls. The rest streams in behind
        # (load_qk_rest after batch 0's first X chunk; wv/wo after
        # phase A of batch 0 is emitted).
        wqk_sb, wqkl_sb, wv_sb, wo_sb = [], [], [], []
        for ct in range(CT):
            t = const.tile([P, 2 * C], BF, tag=f"wqk{ct}")
            nc.sync.dma_start(t[:, 0:512],
                              wqk_p.ap()[ct * P:(ct + 1) * P, 0:512])
            wqk_sb.append(t)
        bqkb = const.tile([P, 2 * C], F32, tag="bqkb")
        bvt = const.tile([P, CT], F32, tag="bvt")
        bot = const.tile([P, CT], F32, tag="bot")
        if split:
            for ct in range(CT):
                t = const.tile([P, 2 * C], BF, tag=f"wqkl{ct}")
                wqkl_sb.append(t)

        def load_qk_rest():
            for ct in range(CT):
                nc.sync.dma_start(wqk_sb[ct][:, 512:2 * C],
                                  wqk_p.ap()[ct * P:(ct + 1) * P, 512:2 * C])
            nc.sync.dma_start(bqkb[:], bqkb_p.ap()[:, :])
            nc.sync.dma_start(bvt[:], bvt_p.ap()[:, :])
            nc.sync.dma_start(bot[:], bot_p.ap()[:, :])
            if split:
                for ct in range(CT):
                    nc.sync.dma_start(wqkl_sb[ct][:],
                                      wqkl_p.ap()[ct * P:(ct + 1) * P, :])

        for name, lst in (("wv", wv_sb), ("wo", wo_sb)):
            for ct in range(CT):
                t = const.tile([P, C], BF, tag=f"{name}{ct}")
                lst.append(t)

        def load_vo_weights():
            for par, lst in ((wvt_p, wv_sb), (wot_p, wo_sb)):
                for ct in range(CT):
                    nc.sync.dma_start(lst[ct][:], par.ap()[ct * P:(ct + 1) * P, :])

        # [128, 64] with eye(64) stacked twice (for per-half PE transposes)
        ident = const.tile([P, 64], BF, tag="ident")
        make_identity(nc, ident[0:64, :])
        make_identity(nc, ident[64:128, :])

        for b in range(NB):
            # --- load X (bf16, [c, n] layout) ---
            # chunked column-major so phase A's first n-tiles (which need
            # ALL six c-tiles' leading columns) arrive first
            xt = [x_pool.tile([P, N], BF, tag="x", name=f"x{b}_{i}")
                  for i in range(CT)]
            bounds = [0, 256, 1536, 2560, N]
            for q in range(4):
                sl = slice(bounds[q], bounds[q + 1])
                for ct in range(CT):
                    nc.sync.dma_start(xt[ct][:, sl],
                                      x_p.ap()[b, ct * P:(ct + 1) * P, sl])
                if b == 0 and q == 0:
                    load_qk_rest()

            def xl_span(q):
                # stream the low half of X in 256-col spans (phase A only)
                tiles = []
                sl = slice(q * 256, (q + 1) * 256)
                for ct in range(CT):
                    t = xl_pool.tile([P, 256], BF, tag="xl",
                                     name=f"xl{b}_{q}_{ct}")
                    nc.sync.dma_start(t[:], xl_p.ap()[b, ct * P:(ct + 1) * P, sl])
                    tiles.append(t)
                return tiles

            # --- Phase A: QT/KT n-tiles + energy accumulation ---
            # E accumulates in SBUF f32: each n-tile's 12 head-matmuls are
            # independent single-shot PSUM groups (disjoint regions, any
            # order), then one DVE add folds the tile into E_sb.
            E_sb = e_pool.tile([P, 64 * NPAIR], F32, tag="Esb")

            def emit_E(qkt, nt):
                eps = psE_pool.tile([P, 64 * NPAIR], F32, tag="E")
                for p in range(NPAIR):
                    for j in range(2):
                        h = 2 * p + j
                        nc.tensor.matmul(
                            eps[64 * j:64 * j + 64, 64 * p:64 * p + 64],
                            lhsT=qkt[:, 64 * h:64 * h + 64],
                            rhs=qkt[:, C + 64 * h:C + 64 * h + 64],
                            start=True, stop=True)
                if nt == 0:
                    nc.vector.tensor_copy(E_sb[:], eps[:])
                else:
                    nc.vector.tensor_add(E_sb[:], E_sb[:], eps[:])

            pending = None
            xl_tiles = None
            for nt in range(NT):
                if split and nt % 2 == 0:
                    xl_tiles = xl_span(nt // 2)
                qkt = qk_pool.tile([P, 2 * C], BF, tag="qkt")
                for third in range(3):
                    cols = slice(third * 512, third * 512 + 512)
                    ps = psA_pool.tile([P, 512], F32, tag="psA")
                    for ct in range(CT):
                        xh = xt[ct][:, nt * P:(nt + 1) * P]
                        nc.tensor.matmul(
                            ps[:], lhsT=xh, rhs=wqk_sb[ct][:, cols],
                            start=(ct == 0), stop=(not split and ct == CT - 1))
                        if split:
                            nc.tensor.matmul(
                                ps[:], lhsT=xh, rhs=wqkl_sb[ct][:, cols],
                                start=False, stop=False)
                            xlo = xl_tiles[ct][:, (nt % 2) * P:(nt % 2 + 1) * P]
                            nc.tensor.matmul(
                                ps[:], lhsT=xlo, rhs=wqk_sb[ct][:, cols],
                                start=False, stop=(ct == CT - 1))
                    nc.vector.tensor_add(qkt[:, cols], ps[:], bqkb[:, cols])
                if pending is not None:
                    emit_E(*pending)
                pending = (qkt, nt)
            emit_E(*pending)
            if b == 0:
                load_vo_weights()

            # --- softmax + A^T per head pair ---
            rinv = rinv_pool.tile([P, NPAIR], F32, tag="rinv")
            at_sb = []
            for p in range(NPAIR):
                esl = E_sb[:, 64 * p:64 * p + 64]
                negmax = stat_pool.tile([P, 1], F32, tag="negmax")
                nc.vector.reduce_max(negmax[:], esl, axis=AX.X, negate=True)
                a_sb = a_pool.tile([P, 64], BF, tag="A")
                ssum = stat_pool.tile([P, 1], F32, tag="ssum")
                nc.scalar.activation(a_sb[:], esl, ACT.Exp,
                                     bias=negmax[:], accum_out=ssum[:])
                nc.vector.reciprocal(rinv[:, p:p + 1], ssum[:])
                psat = psat_pool.tile([P, 64], BF, tag="psat")
                nc.tensor.transpose(psat[0:64, :], a_sb[0:64, :], ident[0:64, :])
                nc.tensor.transpose(psat[64:128, :], a_sb[64:128, :], ident[64:128, :])
                # block-diagonal A^T [128, 128]: one full-width O matmul per
                # chunk computes both heads (zeros kill cross-head terms)
                att = at_pool.tile([P, P], BF, tag="AT")
                nc.gpsimd.memset(att[:], 0.0)
                nc.vector.tensor_copy(att[0:64, 0:64], psat[0:64, :])
                nc.vector.tensor_copy(att[64:128, 64:128], psat[64:128, :])
                at_sb.append(att)

            # --- Phase V/O, software-pipelined by one pair ---
            def emit_v_group(vdst, p, ch):
                sl = slice(ch * 512, ch * 512 + 512)
                ps = ps512_pool.tile([P, 512], F32, tag="ps512",
                                     name=f"psv{b}_{p}_{ch}")
                for ct in range(CT):
                    nc.tensor.matmul(
                        ps[:],
                        lhsT=wv_sb[ct][:, p * P:(p + 1) * P],
                        rhs=xt[ct][:, sl],
                        start=(ct == 0), stop=(ct == CT - 1))
                nc.scalar.add(vdst[:, sl], ps[:], bvt[:, p:p + 1])

            o_tiles = []
            v_cur = v_pool.tile([P, N], BF, tag="V", name=f"v{b}_0")
            for ch in range(NCH):
                emit_v_group(v_cur, 0, ch)
            for p in range(NPAIR):
                v_nxt = None
                if p + 1 < NPAIR:
                    v_nxt = v_pool.tile([P, N], BF, tag="V",
                                        name=f"v{b}_{p + 1}")
                o_sb = o_pool.tile([P, N], BF, tag="O", name=f"o{b}_{p}")
                for ch in range(NCH):
                    if v_nxt is not None:
                        emit_v_group(v_nxt, p + 1, ch)
                    sl = slice(ch * 512, ch * 512 + 512)
                    ps = ps512_pool.tile([P, 512], F32, tag="ps512",
                                         name=f"pso{b}_{p}_{ch}")
                    nc.tensor.matmul(ps[:], lhsT=at_sb[p][:], rhs=v_cur[:, sl])
                    if ch % 2 == 0:
                        nc.scalar.mul(o_sb[:, sl], ps[:], rinv[:, p:p + 1])
                    else:
                        nc.vector.tensor_scalar_mul(o_sb[:, sl], ps[:],
                                                    rinv[:, p:p + 1])
                o_tiles.append(o_sb)
                v_cur = v_nxt

            # --- Phase Y: final projection + bias + DMA out ---
            # whole [128, 4096] row-tile staged in SBUF, two half-row DMAs
            # (per-chunk DMAs cost ~1.8us of sequencer sem overhead each)
            for ot in range(CT):
                y_sb = y_pool.tile([P, N], F32, tag="Y")
                for ch in range(NCH):
                    sl = slice(ch * 512, ch * 512 + 512)
                    ps = ps512_pool.tile([P, 512], F32, tag="ps512")
                    for ct in range(CT):
                        nc.tensor.matmul(
                            ps[:],
                            lhsT=wo_sb[ct][:, ot * P:(ot + 1) * P],
                            rhs=o_tiles[ct][:, sl],
                            start=(ct == 0), stop=(ct == CT - 1))
                    if ch % 2 == 0:
                        nc.scalar.add(y_sb[:, sl], ps[:], bot[:, ot:ot + 1])
                    else:
                        nc.vector.tensor_scalar_add(y_sb[:, sl], ps[:],
                                                    bot[:, ot:ot + 1])
                    if ch % 2 == 1:
                        hl = slice((ch - 1) * 512, (ch + 1) * 512)
                        nc.sync.dma_start(
                            out_p.ap()[b, ot * P:(ot + 1) * P, hl],
                            y_sb[:, hl])

    nc.compile()
    return nc


def _get_nc(precision=None):
    key = f"nc_{precision or PRECISION}"
    if key not in _CACHE:
        _CACHE[key] = _build_nc(precision)
    return _CACHE[key]


def _prep_in_maps(x, wq, bq, wk, bk, wv, bv, wo, bo, precision=None):
    if precision is None:
        precision = PRECISION
    split = precision == "splitqk"
    bf16 = ml_dtypes.bfloat16
    x = np.asarray(x, dtype=np.float32).reshape(B, C, N)
    wq = np.asarray(wq, np.float32); wk = np.asarray(wk, np.float32)
    wv = np.asarray(wv, np.float32); wo = np.asarray(wo, np.float32)
    bq = np.asarray(bq, np.float32); bk = np.asarray(bk, np.float32)
    bv = np.asarray(bv, np.float32); bo = np.asarray(bo, np.float32)

    wqk_f = np.ascontiguousarray(
        np.concatenate([(wq * SCALE).T, wk.T], axis=1))
    wqk = wqk_f.astype(bf16)
    wvt = np.ascontiguousarray(wv.T).astype(bf16)
    wot = np.ascontiguousarray(wo.T).astype(bf16)
    bqk = np.concatenate([bq * SCALE, bk])
    bqkb = np.broadcast_to(bqk, (P, 2 * C)).astype(np.float32).copy()
    bvt = np.ascontiguousarray(bv.reshape(CT, P).T).astype(np.float32)
    bot = np.ascontiguousarray(bo.reshape(CT, P).T).astype(np.float32)

    shared = dict(wqk=wqk, wvt=wvt, wot=wot,
                  bqkb=bqkb, bvt=bvt, bot=bot)
    if split:
        shared["wqkl"] = (wqk_f - wqk.astype(np.float32)).astype(bf16)
    in_maps = []
    for core in range(NCORES):
        xf = np.ascontiguousarray(x[core * NB:(core + 1) * NB])
        xs = xf.astype(bf16)
        m = dict(x=xs, **shared)
        if split:
            m["xl"] = (xf - xs.astype(np.float32)).astype(bf16)
        in_maps.append(m)
    return in_maps


def kernel(x, wq, bq, wk, bk, wv, bv, wo, bo, _trace=False, _trace_kwargs=None):
    from concourse.bass_utils import run_bass_kernel_spmd

    nc = _get_nc()
    in_maps = _prep_in_maps(x, wq, bq, wk, bk, wv, bv, wo, bo)
    res = run_bass_kernel_spmd(nc, in_maps, core_ids=list(range(NCORES)),
                               trace=_trace, **(_trace_kwargs or {}))
    _CACHE["last_results"] = res
    out = np.concatenate([res.results[c]["out"] for c in range(NCORES)], axis=0)
    return out.reshape(B, C, HH, WW).astype(np.float32)



# revision 6
# speedup vs baseline: 1.0516x; 1.0516x over previous
"""Trainium2 Bass kernel for channel-attention (AttnBlock-style, contraction
over spatial axis) distributed over 8 NeuronCores.

Problem (hardcoded shapes):
  x: [16, 768, 64, 64] f32; wq/wk/wv/wo: [768, 768]; bq/bk/bv/bo: [768]
  q = wq@x+bq; k = ...; v = ...   (1x1 conv == per-pixel channel matmul)
  energy[b,h,i,j] = sum_n q[b,h,i,n] * k[b,h,j,n] * scale   (n = 4096 spatial)
  attn = softmax(energy, -1);  out[b,h,i,n] = sum_j attn[i,j] v[b,h,j,n]
  y = wo@out+bo

Sharding: pure data-parallel over batch (16 batches -> 2 per core), weights
replicated. No collectives needed.

Algorithm (Gram-matrix reformulation — the big PE saving):
  E_h = (Wq_h X + bq_h 1^T)(Wk_h X + bk_h 1^T)^T
      = Wq_h G Wk_h^T + (Wq_h s) bk_h^T + bq_h (Wk_h s + n bk_h)^T
  with G = X X^T [768,768] (one projection-sized matmul instead of two
  full Q/K projections), s = X @ 1 (row sums). G and T = G Wk^T are
  evicted as hi+lo bf16 pairs so the fold matmuls lose no precision.
  The bias terms enter the E PSUM accumulation as rank-1 outer-product
  matmuls (1-partition lhsT/rhs).

  O side: out_h = A_h V_h, y = sum_h Wo_h out_h = sum_h N_h V_h with
  N_h^T = A_h Wo_h^T built by one small matmul per head pair
  (lhsT = block-diag A, rhs = wot row-block). attn@V disappears; Y
  accumulates straight from V tiles over the 6 head pairs.

Per-batch phases: G (2 passes x 3 column-chunks from streamed x^T),
T = G Wk^T, E + rank-1 bias terms, [softmax -> NT on vector/scalar
engines overlapping] V = Wv X + bv on PE, then Y = sum NT^T V + bo.
"""

import os
import sys
import numpy as np
import ml_dtypes

if "/opt/trn_rl_repo" not in sys.path:
    sys.path.insert(0, "/opt/trn_rl_repo")

B, C, HH, WW = 16, 768, 64, 64
NUM_HEADS = 12
HEAD_DIM = 64
SCALE = HEAD_DIM ** -0.5
N = HH * WW            # 4096 spatial positions
NCORES = 8
NB = B // NCORES       # batches per core = 2
P = 128
CT = C // P            # 6 channel tiles
NT = N // P            # 32 spatial tiles of 128
NCH = N // 512         # 8 spatial chunks of 512
NPAIR = NUM_HEADS // 2 # 6 head pairs

_CACHE = {}


def _build_nc():
    import concourse.bass as bass
    import concourse.bacc as bacc
    import concourse.mybir as mybir
    from concourse.tile import TileContext
    from contextlib import ExitStack

    BF = mybir.dt.bfloat16
    F32 = mybir.dt.float32
    AX = mybir.AxisListType
    ACT = mybir.ActivationFunctionType

    nc = bacc.Bacc("TRN2", target_bir_lowering=False, debug=False,
                   enable_asserts=False, num_devices=NCORES)

    x_p = nc.declare_dram_parameter("x", [NB, C, N], BF, isOutput=False)
    xT_p = nc.declare_dram_parameter("xT", [NB, N, C], BF, isOutput=False)
    wqk_p = nc.declare_dram_parameter("wqk", [C, 2 * C], BF, isOutput=False)
    wvt_p = nc.declare_dram_parameter("wvt", [C, C], BF, isOutput=False)
    wot_p = nc.declare_dram_parameter("wot", [C, C], BF, isOutput=False)
    bvt_p = nc.declare_dram_parameter("bvt", [P, CT], F32, isOutput=False)
    bot_p = nc.declare_dram_parameter("bot", [P, CT], F32, isOutput=False)
    bqr_p = nc.declare_dram_parameter("bqr", [1, C], BF, isOutput=False)
    bkr_p = nc.declare_dram_parameter("bkr", [1, C], BF, isOutput=False)
    bk4_p = nc.declare_dram_parameter("bk4", [1, C], F32, isOutput=False)
    out_p = nc.declare_dram_parameter("out", [NB, C, N], F32, isOutput=True)

    with TileContext(nc) as tc, ExitStack() as ctx:
        const = ctx.enter_context(tc.tile_pool(name="const", bufs=1))
        x_pool = ctx.enter_context(tc.tile_pool(name="xp", bufs=CT))
        xT_pool = ctx.enter_context(tc.tile_pool(name="xTp", bufs=4))
        g_pool = ctx.enter_context(tc.tile_pool(name="gp", bufs=12))
        t_pool = ctx.enter_context(tc.tile_pool(name="tp", bufs=12))
        v_pool = ctx.enter_context(tc.tile_pool(name="vp", bufs=NPAIR))
        y_pool = ctx.enter_context(tc.tile_pool(name="yp", bufs=2))
        s_pool = ctx.enter_context(tc.tile_pool(name="sp", bufs=8))
        row_pool = ctx.enter_context(tc.tile_pool(name="rowp", bufs=2))
        a_pool = ctx.enter_context(tc.tile_pool(name="ap", bufs=4))
        att_pool = ctx.enter_context(tc.tile_pool(name="attp", bufs=NPAIR))
        stat_pool = ctx.enter_context(tc.tile_pool(name="statp", bufs=4))
        rinv_pool = ctx.enter_context(tc.tile_pool(name="rinvp", bufs=2))
        psM_pool = ctx.enter_context(tc.tile_pool(name="psM", bufs=6, space="PSUM"))
        psS_pool = ctx.enter_context(tc.tile_pool(name="psS", bufs=2, space="PSUM"))

        # --- weights / consts ---
        wqk_sb, wv_sb, wo_sb = [], [], []
        for ct in range(CT):
            wqk_sb.append(const.tile([P, 2 * C], BF, tag=f"wqk{ct}",
                                     name=f"wqk{ct}"))
        for name, lst in (("wv", wv_sb), ("wo", wo_sb)):
            for ct in range(CT):
                lst.append(const.tile([P, C], BF, tag=f"{name}{ct}",
                                      name=f"{name}{ct}"))
        bvt = const.tile([P, CT], F32, tag="bvt")
        bot = const.tile([P, CT], F32, tag="bot")
        bqr = const.tile([1, C], BF, tag="bqr")
        bkr = const.tile([1, C], BF, tag="bkr")
        bk4 = const.tile([1, C], F32, tag="bk4")

        def load_weights():
            for ct in range(CT):
                nc.sync.dma_start(wqk_sb[ct][:], wqk_p.ap()[ct * P:(ct + 1) * P, :])
            for par, lst in ((wvt_p, wv_sb), (wot_p, wo_sb)):
                for ct in range(CT):
                    nc.sync.dma_start(lst[ct][:], par.ap()[ct * P:(ct + 1) * P, :])
            nc.sync.dma_start(bvt[:], bvt_p.ap()[:, :])
            nc.sync.dma_start(bot[:], bot_p.ap()[:, :])
            nc.sync.dma_start(bqr[:], bqr_p.ap()[:, :])
            nc.sync.dma_start(bkr[:], bkr_p.ap()[:, :])
            nc.sync.dma_start(bk4[:], bk4_p.ap()[:, :])

        xT_prefetch = {}

        def xT_fetch(b, nt):
            t = xT_prefetch.pop((b, nt), None)
            if t is None:
                t = xT_pool.tile([P, C], BF, tag="xT", name=f"xT{b}_{nt}")
                nc.sync.dma_start(t[:], xT_p.ap()[b, nt * P:(nt + 1) * P, :])
            return t

        def xT_pre(b, nt):
            t = xT_pool.tile([P, C], BF, tag="xT", name=f"xT{b}_{nt}")
            nc.sync.dma_start(t[:], xT_p.ap()[b, nt * P:(nt + 1) * P, :])
            xT_prefetch[(b, nt)] = t

        for b in range(NB):
            # G pass 0 needs only xT tiles: prime the first few so they are
            # not queued behind the bulk x loads (batch 1's are pre-issued
            # from inside batch 0, see bottom of the loop).
            if b == 0:
                for nt in range(4):
                    xT_pre(0, nt)

            # full-channel x tiles ([c, n] layout; feeds s-reduce + V phase)
            xt = [x_pool.tile([P, N], BF, tag="x", name=f"x{b}_{i}")
                  for i in range(CT)]
            for ct in range(CT):
                nc.sync.dma_start(xt[ct][:], x_p.ap()[b, ct * P:(ct + 1) * P, :])
            if b == 0:
                load_weights()

            # --- G = X X^T, hi/lo bf16, two passes of 3 column-chunks ---
            gh = [None] * CT
            gl = [None] * CT
            for pas in range(2):
                chunks = range(3 * pas, 3 * pas + 3)
                psG = {}
                for c2c in chunks:
                    for hf in range(2):
                        psG[(c2c, hf)] = psM_pool.tile(
                            [P, 512], F32, tag="psM", name=f"psG{b}_{c2c}_{hf}")
                for nt in range(NT):
                    xTt = xT_fetch(b, nt)
                    for c2c in chunks:
                        for hf in range(2):
                            nc.tensor.matmul(
                                psG[(c2c, hf)][:, 0:384],
                                lhsT=xTt[:, c2c * P:(c2c + 1) * P],
                                rhs=xTt[:, hf * 384:hf * 384 + 384],
                                start=(nt == 0), stop=(nt == NT - 1))
                for c2c in chunks:
                    gh[c2c] = g_pool.tile([P, C], BF, tag="g",
                                          name=f"gh{b}_{c2c}")
                    gl[c2c] = g_pool.tile([P, C], BF, tag="g",
                                          name=f"gl{b}_{c2c}")
                    for hf in range(2):
                        sl = slice(hf * 384, hf * 384 + 384)
                        ps = psG[(c2c, hf)][:, 0:384]
                        nc.vector.tensor_copy(gh[c2c][:, sl], ps)
                        nc.vector.tensor_sub(gl[c2c][:, sl], ps, gh[c2c][:, sl])

            # --- row sums s = X @ 1 (bf16 for PE use) ---
            s_bf = []
            for ct in range(CT):
                sf = s_pool.tile([P, 1], F32, tag="sf", name=f"sf{b}_{ct}")
                nc.vector.reduce_sum(sf[:], xt[ct][:, :], axis=AX.X)
                sb = s_pool.tile([P, 1], BF, tag="sb", name=f"sb{b}_{ct}")
                nc.vector.tensor_copy(sb[:], sf[:])
                s_bf.append(sb)

            # --- T = G @ Wk^T (hi/lo G in, hi/lo bf16 out) ---
            th = [None] * CT
            tl = [None] * CT
            for ct in range(CT):
                th[ct] = t_pool.tile([P, C], BF, tag="t", name=f"th{b}_{ct}")
                tl[ct] = t_pool.tile([P, C], BF, tag="t", name=f"tl{b}_{ct}")
                for hf in range(2):
                    sl = slice(hf * 384, hf * 384 + 384)
                    psT = psM_pool.tile([P, 512], F32, tag="psM",
                                        name=f"psT{b}_{ct}_{hf}")
                    i = 0
                    for c2c in range(CT):
                        for g in (gh[c2c], gl[c2c]):
                            nc.tensor.matmul(
                                psT[:, 0:384],
                                lhsT=g[:, ct * P:(ct + 1) * P],
                                rhs=wqk_sb[c2c][:, C + hf * 384:C + hf * 384 + 384],
                                start=(i == 0), stop=(i == 2 * CT - 1))
                            i += 1
                    nc.vector.tensor_copy(th[ct][:, sl], psT[:, 0:384])
                    nc.vector.tensor_sub(tl[ct][:, sl], psT[:, 0:384],
                                         th[ct][:, sl])

            # --- qs/ks rows: qs = (Wq_sc s)^T, ks_eff = (Wk s)^T + n bk ---
            tA = psS_pool.tile([P, 512], F32, tag="psS", name=f"qkA{b}")
            tB = psS_pool.tile([P, 512], F32, tag="psS", name=f"qkB{b}")
            for r, off, wid, dst in ((0, 0, 512, tA), (64, C, 512, tA),
                                     (0, 512, 256, tB), (64, C + 512, 256, tB)):
                for ct in range(CT):
                    nc.tensor.matmul(
                        dst[r:r + 1, 0:wid],
                        lhsT=s_bf[ct][:], rhs=wqk_sb[ct][:, off:off + wid],
                        start=(ct == 0), stop=(ct == CT - 1))
            qsr = row_pool.tile([1, C], BF, tag="qsr", name=f"qsr{b}")
            ksr = row_pool.tile([1, C], BF, tag="ksr", name=f"ksr{b}")
            nc.vector.tensor_copy(qsr[:, 0:512], tA[0:1, 0:512])
            nc.vector.tensor_copy(qsr[:, 512:C], tB[0:1, 0:256])
            nc.vector.tensor_add(ksr[:, 0:512], tA[64:65, 0:512], bk4[:, 0:512])
            nc.vector.tensor_add(ksr[:, 512:C], tB[64:65, 0:256], bk4[:, 512:C])

            # --- E accumulation: per head, Wq_h T_h + rank-1 bias terms ---
            psE = psS_pool.tile([P, 512], F32, tag="psS", name=f"psE{b}")
            for p in range(NPAIR):
                cols = slice(64 * p, 64 * p + 64)
                for j in range(2):
                    h = 2 * p + j
                    rows = slice(64 * j, 64 * j + 64)
                    hsl = slice(64 * h, 64 * h + 64)
                    i = 0
                    for ct in range(CT):
                        for t in (th[ct], tl[ct]):
                            nc.tensor.matmul(
                                psE[rows, cols],
                                lhsT=wqk_sb[ct][:, hsl], rhs=t[:, hsl],
                                start=(i == 0), stop=False)
                            i += 1
                    nc.tensor.matmul(psE[rows, cols], lhsT=qsr[:, hsl],
                                     rhs=bkr[:, hsl], start=False, stop=False)
                    nc.tensor.matmul(psE[rows, cols], lhsT=bqr[:, hsl],
                                     rhs=ksr[:, hsl], start=False, stop=True)

            # --- softmax + NT = (A_norm Wo_h^T)^T per pair ---
            # (emitted before the V phase: the vector/scalar softmax chain
            # overlaps V's PE matmuls; NT's PE matmuls land mid-V)
            rinv = rinv_pool.tile([P, NPAIR], F32, tag="rinv", name=f"ri{b}")
            nt_sb = []
            for p in range(NPAIR):
                esl = psE[:, 64 * p:64 * p + 64]
                negmax = stat_pool.tile([P, 1], F32, tag="negmax")
                nc.vector.reduce_max(negmax[:], esl, axis=AX.X, negate=True)
                a_sb = a_pool.tile([P, 64], BF, tag="A")
                ssum = stat_pool.tile([P, 1], F32, tag="ssum")
                nc.scalar.activation(a_sb[:], esl, ACT.Exp,
                                     bias=negmax[:], accum_out=ssum[:])
                nc.vector.reciprocal(rinv[:, p:p + 1], ssum[:])
                a_n = a_pool.tile([P, 64], BF, tag="An")
                nc.scalar.mul(a_n[:], a_sb[:], rinv[:, p:p + 1])
                att = att_pool.tile([P, P], BF, tag="att", name=f"att{b}_{p}")
                nc.gpsimd.memset(att[:], 0.0)
                nc.vector.tensor_copy(att[0:64, 0:64], a_n[0:64, :])
                nc.vector.tensor_copy(att[64:128, 64:128], a_n[64:128, :])
                ntp = g_pool.tile([P, C], BF, tag="g", name=f"nt{b}_{p}")
                for hf in range(2):
                    sl = slice(hf * 384, hf * 384 + 384)
                    psN = psM_pool.tile([P, 512], F32, tag="psM",
                                        name=f"psN{b}_{p}_{hf}")
                    nc.tensor.matmul(psN[:, 0:384], lhsT=att[:],
                                     rhs=wo_sb[p][:, sl], start=True, stop=True)
                    nc.vector.tensor_copy(ntp[:, sl], psN[:, 0:384])
                nt_sb.append(ntp)

            # batch b+1's first xT tiles: issue before this batch's V/Y
            # DMAs so batch b+1's G phase starts without a DMA stall.
            if b + 1 < NB:
                for nt in range(4):
                    xT_pre(b + 1, nt)

            # --- V = Wv X + bv (per pair, evicted bf16) ---
            v_sb = []
            for p in range(NPAIR):
                vt = v_pool.tile([P, N], BF, tag="V", name=f"v{b}_{p}")
                for ch in range(NCH):
                    sl = slice(ch * 512, ch * 512 + 512)
                    ps = psM_pool.tile([P, 512], F32, tag="psM",
                                       name=f"psv{b}_{p}_{ch}")
                    for ct in range(CT):
                        nc.tensor.matmul(
                            ps[:], lhsT=wv_sb[ct][:, p * P:(p + 1) * P],
                            rhs=xt[ct][:, sl],
                            start=(ct == 0), stop=(ct == CT - 1))
                    if ch % 2 == 0:
                        nc.scalar.add(vt[:, sl], ps[:], bvt[:, p:p + 1])
                    else:
                        nc.vector.tensor_scalar_add(vt[:, sl], ps[:],
                                                    bvt[:, p:p + 1])
                v_sb.append(vt)

            # --- Y = sum_pairs NT^T V + bo ---
            for ot in range(CT):
                osl = slice(ot * P, (ot + 1) * P)
                for half in range(2):
                    y_sb = y_pool.tile([P, 2048], F32, tag="Y",
                                       name=f"y{b}_{ot}_{half}")
                    for c4 in range(4):
                        ch = half * 4 + c4
                        sl = slice(ch * 512, ch * 512 + 512)
                        lsl = slice(c4 * 512, c4 * 512 + 512)
                        ps = psM_pool.tile([P, 512], F32, tag="psM",
                                           name=f"psy{b}_{ot}_{ch}")
                        for p in range(NPAIR):
                            nc.tensor.matmul(
                                ps[:], lhsT=nt_sb[p][:, osl],
                                rhs=v_sb[p][:, sl],
                                start=(p == 0), stop=(p == NPAIR - 1))
                        if c4 % 2 == 0:
                            nc.scalar.add(y_sb[:, lsl], ps[:], bot[:, ot:ot + 1])
                        else:
                            nc.vector.tensor_scalar_add(y_sb[:, lsl], ps[:],
                                                        bot[:, ot:ot + 1])
                        if ot == CT - 1:
                            # last row-tile: fine-grained DMAs shrink the
                            # end-of-kernel drain tail
                            nc.sync.dma_start(out_p.ap()[b, osl, sl],
                                              y_sb[:, lsl])
                    if ot < CT - 1:
                        hsl = slice(half * 2048, half * 2048 + 2048)
                        nc.sync.dma_start(out_p.ap()[b, osl, hsl],
                                          y_sb[:, 0:2048])

    nc.compile()
    return nc


def _get_nc():
    if "nc" not in _CACHE:
        _CACHE["nc"] = _build_nc()
    return _CACHE["nc"]


def _prep_in_maps(x, wq, bq, wk, bk, wv, bv, wo, bo):
    bf16 = ml_dtypes.bfloat16
    x = np.asarray(x, dtype=np.float32).reshape(B, C, N)
    wq = np.asarray(wq, np.float32); wk = np.asarray(wk, np.float32)
    wv = np.asarray(wv, np.float32); wo = np.asarray(wo, np.float32)
    bq = np.asarray(bq, np.float32); bk = np.asarray(bk, np.float32)
    bv = np.asarray(bv, np.float32); bo = np.asarray(bo, np.float32)

    wqk = np.ascontiguousarray(
        np.concatenate([(wq * SCALE).T, wk.T], axis=1)).astype(bf16)
    wvt = np.ascontiguousarray(wv.T).astype(bf16)
    wot = np.ascontiguousarray(wo.T).astype(bf16)
    bvt = np.ascontiguousarray(bv.reshape(CT, P).T).astype(np.float32)
    bot = np.ascontiguousarray(bo.reshape(CT, P).T).astype(np.float32)
    bqr = (bq * SCALE).reshape(1, C).astype(bf16)
    bkr = bk.reshape(1, C).astype(bf16)
    bk4 = (N * bk).reshape(1, C).astype(np.float32)

    shared = dict(wqk=wqk, wvt=wvt, wot=wot, bvt=bvt, bot=bot,
                  bqr=bqr, bkr=bkr, bk4=bk4)
    in_maps = []
    for core in range(NCORES):
        xf = np.ascontiguousarray(x[core * NB:(core + 1) * NB])
        xs = xf.astype(bf16)
        xT = np.ascontiguousarray(xs.transpose(0, 2, 1))
        in_maps.append(dict(x=xs, xT=xT, **shared))
    return in_maps


def kernel(x, wq, bq, wk, bk, wv, bv, wo, bo, _trace=False, _trace_kwargs=None):
    from concourse.bass_utils import run_bass_kernel_spmd

    nc = _get_nc()
    in_maps = _prep_in_maps(x, wq, bq, wk, bk, wv, bv, wo, bo)
    res = run_bass_kernel_spmd(nc, in_maps, core_ids=list(range(NCORES)),
                               trace=_trace, **(_trace_kwargs or {}))
    _CACHE["last_results"] = res
    out = np.concatenate([res.results[c]["out"] for c in range(NCORES)], axis=0)
    return out.reshape(B, C, HH, WW).astype(np.float32)


# revision 13
# speedup vs baseline: 1.0808x; 1.0277x over previous
"""Trainium2 Bass kernel for channel-attention (AttnBlock-style, contraction
over spatial axis) distributed over 8 NeuronCores.

Problem (hardcoded shapes):
  x: [16, 768, 64, 64] f32; wq/wk/wv/wo: [768, 768]; bq/bk/bv/bo: [768]
  q = wq@x+bq; k = ...; v = ...   (1x1 conv == per-pixel channel matmul)
  energy[b,h,i,j] = sum_n q[b,h,i,n] * k[b,h,j,n] * scale   (n = 4096 spatial)
  attn = softmax(energy, -1);  out[b,h,i,n] = sum_j attn[i,j] v[b,h,j,n]
  y = wo@out+bo

Sharding: pure data-parallel over batch (16 batches -> 2 per core), weights
replicated. No collectives needed.

Algorithm (Gram-matrix reformulation — the big PE saving):
  E_h = (Wq_h X + bq_h 1^T)(Wk_h X + bk_h 1^T)^T
      = Wq_h G Wk_h^T + (Wq_h s) bk_h^T + bq_h (Wk_h s + n bk_h)^T
  with G = X X^T [768,768] (one projection-sized matmul instead of two
  full Q/K projections), s = X @ 1 (row sums). G and T = G Wk^T are
  evicted as hi+lo bf16 pairs so the fold matmuls lose no precision.
  The bias terms enter the E PSUM accumulation as rank-1 outer-product
  matmuls (1-partition lhsT/rhs).

  O side: out_h = A_h V_h, y = sum_h Wo_h out_h = sum_h N_h V_h with
  N_h^T = A_h Wo_h^T built by one small matmul per head pair
  (lhsT = block-diag A, rhs = wot row-block). attn@V disappears; Y
  accumulates straight from V tiles over the 6 head pairs.

Per-batch phases: G (2 passes x 3 column-chunks from streamed x^T),
T = G Wk^T, E + rank-1 bias terms, [softmax -> NT on vector/scalar
engines overlapping] V = Wv X + bv on PE, then Y = sum NT^T V + bo.
"""

import os
import sys
import numpy as np
import ml_dtypes

if "/opt/trn_rl_repo" not in sys.path:
    sys.path.insert(0, "/opt/trn_rl_repo")

B, C, HH, WW = 16, 768, 64, 64
NUM_HEADS = 12
HEAD_DIM = 64
SCALE = HEAD_DIM ** -0.5
N = HH * WW            # 4096 spatial positions
NCORES = 8
NB = B // NCORES       # batches per core = 2
P = 128
CT = C // P            # 6 channel tiles
NT = N // P            # 32 spatial tiles of 128
NCH = N // 512         # 8 spatial chunks of 512
NPAIR = NUM_HEADS // 2 # 6 head pairs

_CACHE = {}


def _build_nc():
    import concourse.bass as bass
    import concourse.bacc as bacc
    import concourse.mybir as mybir
    from concourse.tile import TileContext
    from contextlib import ExitStack

    BF = mybir.dt.bfloat16
    F32 = mybir.dt.float32
    AX = mybir.AxisListType
    ACT = mybir.ActivationFunctionType

    nc = bacc.Bacc("TRN2", target_bir_lowering=False, debug=False,
                   enable_asserts=False, num_devices=NCORES)

    x_p = nc.declare_dram_parameter("x", [NB, C, N], BF, isOutput=False)
    xT_p = nc.declare_dram_parameter("xT", [NB, N, C], BF, isOutput=False)
    wqk_p = nc.declare_dram_parameter("wqk", [C, 2 * C], BF, isOutput=False)
    wvt_p = nc.declare_dram_parameter("wvt", [C, C], BF, isOutput=False)
    wot_p = nc.declare_dram_parameter("wot", [C, C], BF, isOutput=False)
    bvt_p = nc.declare_dram_parameter("bvt", [P, CT], F32, isOutput=False)
    bot_p = nc.declare_dram_parameter("bot", [P, CT], F32, isOutput=False)
    bqr_p = nc.declare_dram_parameter("bqr", [1, C], BF, isOutput=False)
    bkr_p = nc.declare_dram_parameter("bkr", [1, C], BF, isOutput=False)
    bk4_p = nc.declare_dram_parameter("bk4", [1, C], F32, isOutput=False)
    out_p = nc.declare_dram_parameter("out", [NB, C, N], F32, isOutput=True)

    with TileContext(nc) as tc, ExitStack() as ctx:
        const = ctx.enter_context(tc.tile_pool(name="const", bufs=1))
        x_pool = ctx.enter_context(tc.tile_pool(name="xp", bufs=CT))
        xT_pool = ctx.enter_context(tc.tile_pool(name="xTp", bufs=4))
        g_pool = ctx.enter_context(tc.tile_pool(name="gp", bufs=12))
        t_pool = ctx.enter_context(tc.tile_pool(name="tp", bufs=12))
        v_pool = ctx.enter_context(tc.tile_pool(name="vp", bufs=NPAIR))
        y_pool = ctx.enter_context(tc.tile_pool(name="yp", bufs=2))
        s_pool = ctx.enter_context(tc.tile_pool(name="sp", bufs=8))
        row_pool = ctx.enter_context(tc.tile_pool(name="rowp", bufs=2))
        a_pool = ctx.enter_context(tc.tile_pool(name="ap", bufs=4))
        att_pool = ctx.enter_context(tc.tile_pool(name="attp", bufs=NPAIR))
        stat_pool = ctx.enter_context(tc.tile_pool(name="statp", bufs=4))
        rinv_pool = ctx.enter_context(tc.tile_pool(name="rinvp", bufs=2))
        psM_pool = ctx.enter_context(tc.tile_pool(name="psM", bufs=6, space="PSUM"))
        psS_pool = ctx.enter_context(tc.tile_pool(name="psS", bufs=2, space="PSUM"))

        # --- weights / consts ---
        wqk_sb, wv_sb, wo_sb = [], [], []
        for ct in range(CT):
            wqk_sb.append(const.tile([P, 2 * C], BF, tag=f"wqk{ct}",
                                     name=f"wqk{ct}"))
        for name, lst in (("wv", wv_sb), ("wo", wo_sb)):
            for ct in range(CT):
                lst.append(const.tile([P, C], BF, tag=f"{name}{ct}",
                                      name=f"{name}{ct}"))
        bvt = const.tile([P, CT], F32, tag="bvt")
        bot = const.tile([P, CT], F32, tag="bot")
        bqr = const.tile([1, C], BF, tag="bqr")
        bkr = const.tile([1, C], BF, tag="bkr")
        bk4 = const.tile([1, C], F32, tag="bk4")

        def load_wqk():
            for ct in range(CT):
                nc.sync.dma_start(wqk_sb[ct][:], wqk_p.ap()[ct * P:(ct + 1) * P, :])

        def load_vo():
            for par, lst in ((wvt_p, wv_sb), (wot_p, wo_sb)):
                for ct in range(CT):
                    nc.sync.dma_start(lst[ct][:], par.ap()[ct * P:(ct + 1) * P, :])
            nc.sync.dma_start(bvt[:], bvt_p.ap()[:, :])
            nc.sync.dma_start(bot[:], bot_p.ap()[:, :])
            nc.sync.dma_start(bqr[:], bqr_p.ap()[:, :])
            nc.sync.dma_start(bkr[:], bkr_p.ap()[:, :])
            nc.sync.dma_start(bk4[:], bk4_p.ap()[:, :])

        xT_prefetch = {}

        def xT_fetch(b, nt):
            t = xT_prefetch.pop((b, nt), None)
            if t is None:
                t = xT_pool.tile([P, C], BF, tag="xT", name=f"xT{b}_{nt}")
                nc.sync.dma_start(t[:], xT_p.ap()[b, nt * P:(nt + 1) * P, :])
            return t

        def xT_pre(b, nt):
            t = xT_pool.tile([P, C], BF, tag="xT", name=f"xT{b}_{nt}")
            nc.sync.dma_start(t[:], xT_p.ap()[b, nt * P:(nt + 1) * P, :])
            xT_prefetch[(b, nt)] = t

        for b in range(NB):
            # G pass 0 needs only xT tiles: prime the first few so they are
            # not queued behind the bulk x loads (batch 1's are pre-issued
            # from inside batch 0, see bottom of the loop).
            if b == 0:
                for nt in range(4):
                    xT_pre(0, nt)

            # wqk first (T phase deadline), then the bulk x tiles
            # ([c, n] layout; feeds s-reduce + V phase). wv/wo stream
            # during G pass 2 so G pass 1's xT stream is not starved.
            if b == 0:
                load_wqk()
            xt = [x_pool.tile([P, N], BF, tag="x", name=f"x{b}_{i}")
                  for i in range(CT)]
            for ct in range(CT):
                nc.sync.dma_start(xt[ct][:], x_p.ap()[b, ct * P:(ct + 1) * P, :])

            # --- G = X X^T, hi/lo bf16, two passes of 3 column-chunks ---
            gh = [None] * CT
            gl = [None] * CT
            for pas in range(2):
                chunks = range(3 * pas, 3 * pas + 3)
                psG = {}
                for c2c in chunks:
                    for hf in range(2):
                        psG[(c2c, hf)] = psM_pool.tile(
                            [P, 512], F32, tag="psM", name=f"psG{b}_{c2c}_{hf}")
                for nt in range(NT):
                    xTt = xT_fetch(b, nt)
                    if b == 0 and pas == 1 and nt == 4:
                        load_vo()
                    for c2c in chunks:
                        for hf in range(2):
                            nc.tensor.matmul(
                                psG[(c2c, hf)][:, 0:384],
                                lhsT=xTt[:, c2c * P:(c2c + 1) * P],
                                rhs=xTt[:, hf * 384:hf * 384 + 384],
                                start=(nt == 0), stop=(nt == NT - 1))
                for c2c in chunks:
                    gh[c2c] = g_pool.tile([P, C], BF, tag="g",
                                          name=f"gh{b}_{c2c}")
                    gl[c2c] = g_pool.tile([P, C], BF, tag="g",
                                          name=f"gl{b}_{c2c}")
                    for hf in range(2):
                        sl = slice(hf * 384, hf * 384 + 384)
                        ps = psG[(c2c, hf)][:, 0:384]
                        nc.scalar.copy(gh[c2c][:, sl], ps)
                        nc.vector.tensor_sub(gl[c2c][:, sl], ps, gh[c2c][:, sl])

            # --- row sums s = X @ 1 (bf16 for PE use) ---
            s_bf = []
            for ct in range(CT):
                sf = s_pool.tile([P, 1], F32, tag="sf", name=f"sf{b}_{ct}")
                nc.vector.reduce_sum(sf[:], xt[ct][:, :], axis=AX.X)
                sb = s_pool.tile([P, 1], BF, tag="sb", name=f"sb{b}_{ct}")
                nc.vector.tensor_copy(sb[:], sf[:])
                s_bf.append(sb)

            # --- T = G @ Wk^T (hi/lo G in, hi/lo bf16 out) ---
            th = [None] * CT
            tl = [None] * CT
            for ct in range(CT):
                th[ct] = t_pool.tile([P, C], BF, tag="t", name=f"th{b}_{ct}")
                tl[ct] = t_pool.tile([P, C], BF, tag="t", name=f"tl{b}_{ct}")
                for hf in range(2):
                    sl = slice(hf * 384, hf * 384 + 384)
                    psT = psM_pool.tile([P, 512], F32, tag="psM",
                                        name=f"psT{b}_{ct}_{hf}")
                    i = 0
                    for c2c in range(CT):
                        for g in (gh[c2c], gl[c2c]):
                            nc.tensor.matmul(
                                psT[:, 0:384],
                                lhsT=g[:, ct * P:(ct + 1) * P],
                                rhs=wqk_sb[c2c][:, C + hf * 384:C + hf * 384 + 384],
                                start=(i == 0), stop=(i == 2 * CT - 1))
                            i += 1
                    nc.scalar.copy(th[ct][:, sl], psT[:, 0:384])
                    nc.vector.tensor_sub(tl[ct][:, sl], psT[:, 0:384],
                                         th[ct][:, sl])

            # --- qs/ks rows: qs = (Wq_sc s)^T, ks_eff = (Wk s)^T + n bk ---
            tA = psS_pool.tile([P, 512], F32, tag="psS", name=f"qkA{b}")
            tB = psS_pool.tile([P, 512], F32, tag="psS", name=f"qkB{b}")
            for r, off, wid, dst in ((0, 0, 512, tA), (64, C, 512, tA),
                                     (0, 512, 256, tB), (64, C + 512, 256, tB)):
                for ct in range(CT):
                    nc.tensor.matmul(
                        dst[r:r + 1, 0:wid],
                        lhsT=s_bf[ct][:], rhs=wqk_sb[ct][:, off:off + wid],
                        start=(ct == 0), stop=(ct == CT - 1))
            qsr = row_pool.tile([1, C], BF, tag="qsr", name=f"qsr{b}")
            ksr = row_pool.tile([1, C], BF, tag="ksr", name=f"ksr{b}")
            nc.vector.tensor_copy(qsr[:, 0:512], tA[0:1, 0:512])
            nc.vector.tensor_copy(qsr[:, 512:C], tB[0:1, 0:256])
            nc.vector.tensor_add(ksr[:, 0:512], tA[64:65, 0:512], bk4[:, 0:512])
            nc.vector.tensor_add(ksr[:, 512:C], tB[64:65, 0:256], bk4[:, 512:C])

            # --- E accumulation: per head, Wq_h T_h + rank-1 bias terms ---
            psE = psS_pool.tile([P, 512], F32, tag="psS", name=f"psE{b}")
            for p in range(NPAIR):
                cols = slice(64 * p, 64 * p + 64)
                for j in range(2):
                    h = 2 * p + j
                    rows = slice(64 * j, 64 * j + 64)
                    hsl = slice(64 * h, 64 * h + 64)
                    i = 0
                    for ct in range(CT):
                        for t in (th[ct], tl[ct]):
                            nc.tensor.matmul(
                                psE[rows, cols],
                                lhsT=wqk_sb[ct][:, hsl], rhs=t[:, hsl],
                                start=(i == 0), stop=False)
                            i += 1
                    nc.tensor.matmul(psE[rows, cols], lhsT=qsr[:, hsl],
                                     rhs=bkr[:, hsl], start=False, stop=False)
                    nc.tensor.matmul(psE[rows, cols], lhsT=bqr[:, hsl],
                                     rhs=ksr[:, hsl], start=False, stop=True)

            # --- V = Wv X + bv (per pair, evicted bf16) ---
            # Emitted before softmax+NT: the PE chews through V's matmuls
            # while the vector/scalar softmax chain runs, and NT's PE
            # matmuls then start without waiting on softmax.
            v_sb = []
            for p in range(NPAIR):
                vt = v_pool.tile([P, N], BF, tag="V", name=f"v{b}_{p}")
                for ch in range(NCH):
                    sl = slice(ch * 512, ch * 512 + 512)
                    ps = psM_pool.tile([P, 512], F32, tag="psM",
                                       name=f"psv{b}_{p}_{ch}")
                    for ct in range(CT):
                        nc.tensor.matmul(
                            ps[:], lhsT=wv_sb[ct][:, p * P:(p + 1) * P],
                            rhs=xt[ct][:, sl],
                            start=(ct == 0), stop=(ct == CT - 1))
                    if ch % 2 == 0:
                        nc.scalar.add(vt[:, sl], ps[:], bvt[:, p:p + 1])
                    else:
                        nc.vector.tensor_scalar_add(vt[:, sl], ps[:],
                                                    bvt[:, p:p + 1])
                v_sb.append(vt)

            # --- softmax + NT = (A_norm Wo_h^T)^T per pair ---
            rinv = rinv_pool.tile([P, NPAIR], F32, tag="rinv", name=f"ri{b}")
            nt_sb = []
            for p in range(NPAIR):
                esl = psE[:, 64 * p:64 * p + 64]
                negmax = stat_pool.tile([P, 1], F32, tag="negmax")
                nc.vector.reduce_max(negmax[:], esl, axis=AX.X, negate=True)
                a_sb = a_pool.tile([P, 64], BF, tag="A")
                ssum = stat_pool.tile([P, 1], F32, tag="ssum")
                nc.scalar.activation(a_sb[:], esl, ACT.Exp,
                                     bias=negmax[:], accum_out=ssum[:])
                nc.vector.reciprocal(rinv[:, p:p + 1], ssum[:])
                a_n = a_pool.tile([P, 64], BF, tag="An")
                nc.scalar.mul(a_n[:], a_sb[:], rinv[:, p:p + 1])
                att = att_pool.tile([P, P], BF, tag="att", name=f"att{b}_{p}")
                nc.gpsimd.memset(att[:], 0.0)
                nc.vector.tensor_copy(att[0:64, 0:64], a_n[0:64, :])
                nc.vector.tensor_copy(att[64:128, 64:128], a_n[64:128, :])
                ntp = g_pool.tile([P, C], BF, tag="g", name=f"nt{b}_{p}")
                for hf in range(2):
                    sl = slice(hf * 384, hf * 384 + 384)
                    psN = psM_pool.tile([P, 512], F32, tag="psM",
                                        name=f"psN{b}_{p}_{hf}")
                    nc.tensor.matmul(psN[:, 0:384], lhsT=att[:],
                                     rhs=wo_sb[p][:, sl], start=True, stop=True)
                    nc.vector.tensor_copy(ntp[:, sl], psN[:, 0:384])
                nt_sb.append(ntp)

            # batch b+1's first xT tiles: issue before this batch's V/Y
            # DMAs so batch b+1's G phase starts without a DMA stall.
            if b + 1 < NB:
                for nt in range(4):
                    xT_pre(b + 1, nt)

            # --- Y = sum_pairs NT^T V + bo ---
            for ot in range(CT):
                osl = slice(ot * P, (ot + 1) * P)
                for half in range(2):
                    y_sb = y_pool.tile([P, 2048], F32, tag="Y",
                                       name=f"y{b}_{ot}_{half}")
                    for c4 in range(4):
                        ch = half * 4 + c4
                        sl = slice(ch * 512, ch * 512 + 512)
                        lsl = slice(c4 * 512, c4 * 512 + 512)
                        ps = psM_pool.tile([P, 512], F32, tag="psM",
                                           name=f"psy{b}_{ot}_{ch}")
                        for p in range(NPAIR):
                            nc.tensor.matmul(
                                ps[:], lhsT=nt_sb[p][:, osl],
                                rhs=v_sb[p][:, sl],
                                start=(p == 0), stop=(p == NPAIR - 1))
                        if c4 % 2 == 0:
                            nc.scalar.add(y_sb[:, lsl], ps[:], bot[:, ot:ot + 1])
                        else:
                            nc.vector.tensor_scalar_add(y_sb[:, lsl], ps[:],
                                                        bot[:, ot:ot + 1])
                        if ot == CT - 1:
                            # last row-tile: fine-grained DMAs shrink the
                            # end-of-kernel drain tail
                            nc.sync.dma_start(out_p.ap()[b, osl, sl],
                                              y_sb[:, lsl])
                    if ot < CT - 1:
                        hsl = slice(half * 2048, half * 2048 + 2048)
                        nc.sync.dma_start(out_p.ap()[b, osl, hsl],
                                          y_sb[:, 0:2048])

    nc.compile()
    return nc


def _get_nc():
    if "nc" not in _CACHE:
        _CACHE["nc"] = _build_nc()
    return _CACHE["nc"]


def _prep_in_maps(x, wq, bq, wk, bk, wv, bv, wo, bo):
    bf16 = ml_dtypes.bfloat16
    x = np.asarray(x, dtype=np.float32).reshape(B, C, N)
    wq = np.asarray(wq, np.float32); wk = np.asarray(wk, np.float32)
    wv = np.asarray(wv, np.float32); wo = np.asarray(wo, np.float32)
    bq = np.asarray(bq, np.float32); bk = np.asarray(bk, np.float32)
    bv = np.asarray(bv, np.float32); bo = np.asarray(bo, np.float32)

    wqk = np.ascontiguousarray(
        np.concatenate([(wq * SCALE).T, wk.T], axis=1)).astype(bf16)
    wvt = np.ascontiguousarray(wv.T).astype(bf16)
    wot = np.ascontiguousarray(wo.T).astype(bf16)
    bvt = np.ascontiguousarray(bv.reshape(CT, P).T).astype(np.float32)
    bot = np.ascontiguousarray(bo.reshape(CT, P).T).astype(np.float32)
    bqr = (bq * SCALE).reshape(1, C).astype(bf16)
    bkr = bk.reshape(1, C).astype(bf16)
    bk4 = (N * bk).reshape(1, C).astype(np.float32)

    shared = dict(wqk=wqk, wvt=wvt, wot=wot, bvt=bvt, bot=bot,
                  bqr=bqr, bkr=bkr, bk4=bk4)
    in_maps = []
    for core in range(NCORES):
        xf = np.ascontiguousarray(x[core * NB:(core + 1) * NB])
        xs = xf.astype(bf16)
        xT = np.ascontiguousarray(xs.transpose(0, 2, 1))
        in_maps.append(dict(x=xs, xT=xT, **shared))
    return in_maps


def kernel(x, wq, bq, wk, bk, wv, bv, wo, bo, _trace=False, _trace_kwargs=None):
    from concourse.bass_utils import run_bass_kernel_spmd

    nc = _get_nc()
    in_maps = _prep_in_maps(x, wq, bq, wk, bk, wv, bv, wo, bo)
    res = run_bass_kernel_spmd(nc, in_maps, core_ids=list(range(NCORES)),
                               trace=_trace, **(_trace_kwargs or {}))
    _CACHE["last_results"] = res
    out = np.concatenate([res.results[c]["out"] for c in range(NCORES)], axis=0)
    return out.reshape(B, C, HH, WW).astype(np.float32)


# revision 20
# speedup vs baseline: 1.2175x; 1.1265x over previous
"""Trainium2 Bass kernel for channel-attention (AttnBlock-style, contraction
over spatial axis) distributed over 8 NeuronCores.

Problem (hardcoded shapes):
  x: [16, 768, 64, 64] f32; wq/wk/wv/wo: [768, 768]; bq/bk/bv/bo: [768]
  q = wq@x+bq; k = ...; v = ...   (1x1 conv == per-pixel channel matmul)
  energy[b,h,i,j] = sum_n q[b,h,i,n] * k[b,h,j,n] * scale   (n = 4096 spatial)
  attn = softmax(energy, -1);  out[b,h,i,n] = sum_j attn[i,j] v[b,h,j,n]
  y = wo@out+bo

Sharding: pure data-parallel over batch (16 batches -> 2 per core), weights
replicated. No collectives needed.

Algorithm (Gram-matrix reformulation — the big PE saving):
  E_h = (Wq_h X + bq_h 1^T)(Wk_h X + bk_h 1^T)^T
      = Wq_h G Wk_h^T + (Wq_h s) bk_h^T + bq_h (Wk_h s + n bk_h)^T
  with G = X X^T [768,768] (one projection-sized matmul instead of two
  full Q/K projections), s = X @ 1 (row sums). G and T = G Wk^T are
  evicted as hi+lo bf16 pairs so the fold matmuls lose no precision.
  The bias terms enter the E PSUM accumulation as rank-1 outer-product
  matmuls (1-partition lhsT/rhs).

  O side: out_h = A_h V_h, y = sum_h Wo_h out_h = sum_h N_h V_h with
  N_h^T = A_h Wo_h^T built by one small matmul per head pair
  (lhsT = block-diag A, rhs = wot row-block). attn@V disappears; Y
  accumulates straight from V tiles over the 6 head pairs.

Per-batch phases: G (2 passes x 3 column-chunks from streamed x^T),
T = G Wk^T, E + rank-1 bias terms, [softmax -> NT on vector/scalar
engines overlapping] V = Wv X + bv on PE, then Y = sum NT^T V + bo.
"""

import os
import sys
import numpy as np
import ml_dtypes

if "/opt/trn_rl_repo" not in sys.path:
    sys.path.insert(0, "/opt/trn_rl_repo")

B, C, HH, WW = 16, 768, 64, 64
NUM_HEADS = 12
HEAD_DIM = 64
SCALE = HEAD_DIM ** -0.5
N = HH * WW            # 4096 spatial positions
NCORES = 8
NB = B // NCORES       # batches per core = 2
P = 128
CT = C // P            # 6 channel tiles
NT = N // P            # 32 spatial tiles of 128
NCH = N // 512         # 8 spatial chunks of 512
NPAIR = NUM_HEADS // 2 # 6 head pairs

_CACHE = {}


def _build_nc():
    import concourse.bass as bass
    import concourse.bacc as bacc
    import concourse.mybir as mybir
    from concourse.tile import TileContext
    from contextlib import ExitStack

    BF = mybir.dt.bfloat16
    F32 = mybir.dt.float32
    AX = mybir.AxisListType
    ACT = mybir.ActivationFunctionType

    nc = bacc.Bacc("TRN2", target_bir_lowering=False, debug=False,
                   enable_asserts=False, num_devices=NCORES)

    x_p = nc.declare_dram_parameter("x", [NB, C, N], BF, isOutput=False)
    xT_p = nc.declare_dram_parameter("xT", [NB, N, C], BF, isOutput=False)
    wqk_p = nc.declare_dram_parameter("wqk", [C, 2 * C], BF, isOutput=False)
    wvt_p = nc.declare_dram_parameter("wvt", [C, C], BF, isOutput=False)
    wot_p = nc.declare_dram_parameter("wot", [C, C], BF, isOutput=False)
    bvt_p = nc.declare_dram_parameter("bvt", [P, CT], F32, isOutput=False)
    bot_p = nc.declare_dram_parameter("bot", [P, CT], F32, isOutput=False)
    bqr_p = nc.declare_dram_parameter("bqr", [1, C], BF, isOutput=False)
    bkr_p = nc.declare_dram_parameter("bkr", [1, C], BF, isOutput=False)
    bk4_p = nc.declare_dram_parameter("bk4", [1, C], F32, isOutput=False)
    out_p = nc.declare_dram_parameter("out", [NB, C, N], F32, isOutput=True)

    with TileContext(nc) as tc, ExitStack() as ctx:
        const = ctx.enter_context(tc.tile_pool(name="const", bufs=1))
        x_pool = ctx.enter_context(tc.tile_pool(name="xp", bufs=CT))
        xT_pool = ctx.enter_context(tc.tile_pool(name="xTp", bufs=4))
        g_pool = ctx.enter_context(tc.tile_pool(name="gp", bufs=12))
        t_pool = ctx.enter_context(tc.tile_pool(name="tp", bufs=12))
        v_pool = ctx.enter_context(tc.tile_pool(name="vp", bufs=NPAIR))
        y_pool = ctx.enter_context(tc.tile_pool(name="yp", bufs=2))
        s_pool = ctx.enter_context(tc.tile_pool(name="sp", bufs=8))
        row_pool = ctx.enter_context(tc.tile_pool(name="rowp", bufs=2))
        a_pool = ctx.enter_context(tc.tile_pool(name="ap", bufs=4))
        att_pool = ctx.enter_context(tc.tile_pool(name="attp", bufs=NPAIR))
        stat_pool = ctx.enter_context(tc.tile_pool(name="statp", bufs=4))
        rinv_pool = ctx.enter_context(tc.tile_pool(name="rinvp", bufs=2))
        psM_pool = ctx.enter_context(tc.tile_pool(name="psM", bufs=6, space="PSUM"))
        psS_pool = ctx.enter_context(tc.tile_pool(name="psS", bufs=2, space="PSUM"))

        # --- weights / consts ---
        wqk_sb, wv_sb, wo_sb = [], [], []
        for ct in range(CT):
            wqk_sb.append(const.tile([P, 2 * C], BF, tag=f"wqk{ct}",
                                     name=f"wqk{ct}"))
        for name, lst in (("wv", wv_sb), ("wo", wo_sb)):
            for ct in range(CT):
                lst.append(const.tile([P, C], BF, tag=f"{name}{ct}",
                                      name=f"{name}{ct}"))
        bvt = const.tile([P, CT], F32, tag="bvt")
        bot = const.tile([P, CT], F32, tag="bot")
        bqr = const.tile([1, C], BF, tag="bqr")
        bkr = const.tile([1, C], BF, tag="bkr")
        bk4 = const.tile([1, C], F32, tag="bk4")

        def load_wqk():
            for ct in range(CT):
                nc.sync.dma_start(wqk_sb[ct][:], wqk_p.ap()[ct * P:(ct + 1) * P, :])

        def load_vo():
            for par, lst in ((wvt_p, wv_sb), (wot_p, wo_sb)):
                for ct in range(CT):
                    nc.sync.dma_start(lst[ct][:], par.ap()[ct * P:(ct + 1) * P, :])
            nc.sync.dma_start(bvt[:], bvt_p.ap()[:, :])
            nc.sync.dma_start(bot[:], bot_p.ap()[:, :])
            nc.sync.dma_start(bqr[:], bqr_p.ap()[:, :])
            nc.sync.dma_start(bkr[:], bkr_p.ap()[:, :])
            nc.sync.dma_start(bk4[:], bk4_p.ap()[:, :])

        # x^T is staged fully in SBUF (borrowing the V pool's big buffers,
        # which are free until the V phase) so G's two passes read it with
        # a single DMA stream. nt 0/1 live in the small ring pool so the
        # next batch's G can start before this batch's Y drains V.
        xT_small = {}

        def xT_pre(b, nt):
            t = xT_pool.tile([P, C], BF, tag="xT", name=f"xT{b}_{nt}")
            nc.sync.dma_start(t[:], xT_p.ap()[b, nt * P:(nt + 1) * P, :])
            xT_small[(b, nt)] = t

        for b in range(NB):
            if b == 0:
                for nt in range(2):
                    xT_pre(0, nt)

            xtb = [v_pool.tile([P, N], BF, tag="V", name=f"xtb{b}_{j}")
                   for j in range(CT)]
            for nt in range(2, NT):
                j, k = (nt - 2) // 5, (nt - 2) % 5
                nc.sync.dma_start(xtb[j][:, k * C:(k + 1) * C],
                                  xT_p.ap()[b, nt * P:(nt + 1) * P, :])
                if b == 0 and nt == 13:
                    load_wqk()

            def xT_at(nt, b=b, xtb=xtb):
                t = xT_small.get((b, nt))
                if t is not None:
                    return t, 0
                j, k = (nt - 2) // 5, (nt - 2) % 5
                return xtb[j], k * C

            xt = [x_pool.tile([P, N], BF, tag="x", name=f"x{b}_{i}")
                  for i in range(CT)]
            for ct in range(CT):
                nc.sync.dma_start(xt[ct][:], x_p.ap()[b, ct * P:(ct + 1) * P, :])
            if b == 0:
                load_vo()

            # --- G = X X^T, hi/lo bf16, two passes of 3 column-chunks ---
            gh = [None] * CT
            gl = [None] * CT
            for pas in range(2):
                chunks = range(3 * pas, 3 * pas + 3)
                psG = {}
                for c2c in chunks:
                    for hf in range(2):
                        psG[(c2c, hf)] = psM_pool.tile(
                            [P, 512], F32, tag="psM", name=f"psG{b}_{c2c}_{hf}")
                for nt in range(NT):
                    xTt, o = xT_at(nt)
                    for c2c in chunks:
                        for hf in range(2):
                            nc.tensor.matmul(
                                psG[(c2c, hf)][:, 0:384],
                                lhsT=xTt[:, o + c2c * P:o + (c2c + 1) * P],
                                rhs=xTt[:, o + hf * 384:o + hf * 384 + 384],
                                start=(nt == 0), stop=(nt == NT - 1))
                for c2c in chunks:
                    gh[c2c] = g_pool.tile([P, C], BF, tag="g",
                                          name=f"gh{b}_{c2c}")
                    gl[c2c] = g_pool.tile([P, C], BF, tag="g",
                                          name=f"gl{b}_{c2c}")
                    for hf in range(2):
                        sl = slice(hf * 384, hf * 384 + 384)
                        ps = psG[(c2c, hf)][:, 0:384]
                        nc.scalar.copy(gh[c2c][:, sl], ps)
                        nc.vector.tensor_sub(gl[c2c][:, sl], ps, gh[c2c][:, sl])

            # --- row sums s = X @ 1 (bf16 for PE use) ---
            s_bf = []
            for ct in range(CT):
                sf = s_pool.tile([P, 1], F32, tag="sf", name=f"sf{b}_{ct}")
                nc.vector.reduce_sum(sf[:], xt[ct][:, :], axis=AX.X)
                sb = s_pool.tile([P, 1], BF, tag="sb", name=f"sb{b}_{ct}")
                nc.vector.tensor_copy(sb[:], sf[:])
                s_bf.append(sb)

            # --- T = G @ Wk^T (hi/lo G in, hi/lo bf16 out) ---
            th = [None] * CT
            tl = [None] * CT
            for ct in range(CT):
                th[ct] = t_pool.tile([P, C], BF, tag="t", name=f"th{b}_{ct}")
                tl[ct] = t_pool.tile([P, C], BF, tag="t", name=f"tl{b}_{ct}")
                for hf in range(2):
                    sl = slice(hf * 384, hf * 384 + 384)
                    psT = psM_pool.tile([P, 512], F32, tag="psM",
                                        name=f"psT{b}_{ct}_{hf}")
                    i = 0
                    for c2c in range(CT):
                        for g in (gh[c2c], gl[c2c]):
                            nc.tensor.matmul(
                                psT[:, 0:384],
                                lhsT=g[:, ct * P:(ct + 1) * P],
                                rhs=wqk_sb[c2c][:, C + hf * 384:C + hf * 384 + 384],
                                start=(i == 0), stop=(i == 2 * CT - 1))
                            i += 1
                    nc.scalar.copy(th[ct][:, sl], psT[:, 0:384])
                    nc.vector.tensor_sub(tl[ct][:, sl], psT[:, 0:384],
                                         th[ct][:, sl])

            # --- qs/ks rows: qs = (Wq_sc s)^T, ks_eff = (Wk s)^T + n bk ---
            tA = psS_pool.tile([P, 512], F32, tag="psS", name=f"qkA{b}")
            tB = psS_pool.tile([P, 512], F32, tag="psS", name=f"qkB{b}")
            for r, off, wid, dst in ((0, 0, 512, tA), (64, C, 512, tA),
                                     (0, 512, 256, tB), (64, C + 512, 256, tB)):
                for ct in range(CT):
                    nc.tensor.matmul(
                        dst[r:r + 1, 0:wid],
                        lhsT=s_bf[ct][:], rhs=wqk_sb[ct][:, off:off + wid],
                        start=(ct == 0), stop=(ct == CT - 1))
            qsr = row_pool.tile([1, C], BF, tag="qsr", name=f"qsr{b}")
            ksr = row_pool.tile([1, C], BF, tag="ksr", name=f"ksr{b}")
            nc.vector.tensor_copy(qsr[:, 0:512], tA[0:1, 0:512])
            nc.vector.tensor_copy(qsr[:, 512:C], tB[0:1, 0:256])
            nc.vector.tensor_add(ksr[:, 0:512], tA[64:65, 0:512], bk4[:, 0:512])
            nc.vector.tensor_add(ksr[:, 512:C], tB[64:65, 0:256], bk4[:, 512:C])

            # --- E accumulation: per head, Wq_h T_h + rank-1 bias terms ---
            psE = psS_pool.tile([P, 512], F32, tag="psS", name=f"psE{b}")
            for p in range(NPAIR):
                cols = slice(64 * p, 64 * p + 64)
                for j in range(2):
                    h = 2 * p + j
                    rows = slice(64 * j, 64 * j + 64)
                    hsl = slice(64 * h, 64 * h + 64)
                    i = 0
                    for ct in range(CT):
                        for t in (th[ct], tl[ct]):
                            nc.tensor.matmul(
                                psE[rows, cols],
                                lhsT=wqk_sb[ct][:, hsl], rhs=t[:, hsl],
                                start=(i == 0), stop=False)
                            i += 1
                    nc.tensor.matmul(psE[rows, cols], lhsT=qsr[:, hsl],
                                     rhs=bkr[:, hsl], start=False, stop=False)
                    nc.tensor.matmul(psE[rows, cols], lhsT=bqr[:, hsl],
                                     rhs=ksr[:, hsl], start=False, stop=True)

            # --- V = Wv X + bv (per pair, evicted bf16) ---
            # Emitted before softmax+NT: the PE chews through V's matmuls
            # while the vector/scalar softmax chain runs, and NT's PE
            # matmuls then start without waiting on softmax.
            v_sb = []
            for p in range(NPAIR):
                vt = v_pool.tile([P, N], BF, tag="V", name=f"v{b}_{p}")
                for ch in range(NCH):
                    sl = slice(ch * 512, ch * 512 + 512)
                    ps = psM_pool.tile([P, 512], F32, tag="psM",
                                       name=f"psv{b}_{p}_{ch}")
                    for ct in range(CT):
                        nc.tensor.matmul(
                            ps[:], lhsT=wv_sb[ct][:, p * P:(p + 1) * P],
                            rhs=xt[ct][:, sl],
                            start=(ct == 0), stop=(ct == CT - 1))
                    if ch % 2 == 0:
                        nc.scalar.add(vt[:, sl], ps[:], bvt[:, p:p + 1])
                    else:
                        nc.vector.tensor_scalar_add(vt[:, sl], ps[:],
                                                    bvt[:, p:p + 1])
                v_sb.append(vt)

            # --- softmax + NT = (A_norm Wo_h^T)^T per pair ---
            rinv = rinv_pool.tile([P, NPAIR], F32, tag="rinv", name=f"ri{b}")
            nt_sb = []
            for p in range(NPAIR):
                esl = psE[:, 64 * p:64 * p + 64]
                negmax = stat_pool.tile([P, 1], F32, tag="negmax")
                nc.vector.reduce_max(negmax[:], esl, axis=AX.X, negate=True)
                a_sb = a_pool.tile([P, 64], BF, tag="A")
                ssum = stat_pool.tile([P, 1], F32, tag="ssum")
                nc.scalar.activation(a_sb[:], esl, ACT.Exp,
                                     bias=negmax[:], accum_out=ssum[:])
                nc.vector.reciprocal(rinv[:, p:p + 1], ssum[:])
                a_n = a_pool.tile([P, 64], BF, tag="An")
                nc.scalar.mul(a_n[:], a_sb[:], rinv[:, p:p + 1])
                att = att_pool.tile([P, P], BF, tag="att", name=f"att{b}_{p}")
                nc.gpsimd.memset(att[:], 0.0)
                nc.vector.tensor_copy(att[0:64, 0:64], a_n[0:64, :])
                nc.vector.tensor_copy(att[64:128, 64:128], a_n[64:128, :])
                ntp = g_pool.tile([P, C], BF, tag="g", name=f"nt{b}_{p}")
                for hf in range(2):
                    sl = slice(hf * 384, hf * 384 + 384)
                    psN = psM_pool.tile([P, 512], F32, tag="psM",
                                        name=f"psN{b}_{p}_{hf}")
                    nc.tensor.matmul(psN[:, 0:384], lhsT=att[:],
                                     rhs=wo_sb[p][:, sl], start=True, stop=True)
                    nc.vector.tensor_copy(ntp[:, sl], psN[:, 0:384])
                nt_sb.append(ntp)

            # batch b+1's first xT tiles: issue before this batch's V/Y
            # DMAs so batch b+1's G phase starts without a DMA stall.
            if b + 1 < NB:
                for nt in range(2):
                    xT_pre(b + 1, nt)

            # --- Y = sum_pairs NT^T V + bo ---
            for ot in range(CT):
                osl = slice(ot * P, (ot + 1) * P)
                for half in range(2):
                    y_sb = y_pool.tile([P, 2048], F32, tag="Y",
                                       name=f"y{b}_{ot}_{half}")
                    for c4 in range(4):
                        ch = half * 4 + c4
                        sl = slice(ch * 512, ch * 512 + 512)
                        lsl = slice(c4 * 512, c4 * 512 + 512)
                        ps = psM_pool.tile([P, 512], F32, tag="psM",
                                           name=f"psy{b}_{ot}_{ch}")
                        for p in range(NPAIR):
                            nc.tensor.matmul(
                                ps[:], lhsT=nt_sb[p][:, osl],
                                rhs=v_sb[p][:, sl],
                                start=(p == 0), stop=(p == NPAIR - 1))
                        if c4 % 2 == 0:
                            nc.scalar.add(y_sb[:, lsl], ps[:], bot[:, ot:ot + 1])
                        else:
                            nc.vector.tensor_scalar_add(y_sb[:, lsl], ps[:],
                                                        bot[:, ot:ot + 1])
                        if ot == CT - 1:
                            # last row-tile: fine-grained DMAs shrink the
                            # end-of-kernel drain tail
                            nc.sync.dma_start(out_p.ap()[b, osl, sl],
                                              y_sb[:, lsl])
                    if ot < CT - 1:
                        hsl = slice(half * 2048, half * 2048 + 2048)
                        nc.sync.dma_start(out_p.ap()[b, osl, hsl],
                                          y_sb[:, 0:2048])

    nc.compile()
    return nc


def _get_nc():
    if "nc" not in _CACHE:
        _CACHE["nc"] = _build_nc()
    return _CACHE["nc"]


def _prep_in_maps(x, wq, bq, wk, bk, wv, bv, wo, bo):
    bf16 = ml_dtypes.bfloat16
    x = np.asarray(x, dtype=np.float32).reshape(B, C, N)
    wq = np.asarray(wq, np.float32); wk = np.asarray(wk, np.float32)
    wv = np.asarray(wv, np.float32); wo = np.asarray(wo, np.float32)
    bq = np.asarray(bq, np.float32); bk = np.asarray(bk, np.float32)
    bv = np.asarray(bv, np.float32); bo = np.asarray(bo, np.float32)

    wqk = np.ascontiguousarray(
        np.concatenate([(wq * SCALE).T, wk.T], axis=1)).astype(bf16)
    wvt = np.ascontiguousarray(wv.T).astype(bf16)
    wot = np.ascontiguousarray(wo.T).astype(bf16)
    bvt = np.ascontiguousarray(bv.reshape(CT, P).T).astype(np.float32)
    bot = np.ascontiguousarray(bo.reshape(CT, P).T).astype(np.float32)
    bqr = (bq * SCALE).reshape(1, C).astype(bf16)
    bkr = bk.reshape(1, C).astype(bf16)
    bk4 = (N * bk).reshape(1, C).astype(np.float32)

    shared = dict(wqk=wqk, wvt=wvt, wot=wot, bvt=bvt, bot=bot,
                  bqr=bqr, bkr=bkr, bk4=bk4)
    in_maps = []
    for core in range(NCORES):
        xf = np.ascontiguousarray(x[core * NB:(core + 1) * NB])
        xs = xf.astype(bf16)
        xT = np.ascontiguousarray(xs.transpose(0, 2, 1))
        in_maps.append(dict(x=xs, xT=xT, **shared))
    return in_maps


def kernel(x, wq, bq, wk, bk, wv, bv, wo, bo, _trace=False, _trace_kwargs=None):
    from concourse.bass_utils import run_bass_kernel_spmd

    nc = _get_nc()
    in_maps = _prep_in_maps(x, wq, bq, wk, bk, wv, bv, wo, bo)
    res = run_bass_kernel_spmd(nc, in_maps, core_ids=list(range(NCORES)),
                               trace=_trace, **(_trace_kwargs or {}))
    _CACHE["last_results"] = res
    out = np.concatenate([res.results[c]["out"] for c in range(NCORES)], axis=0)
    return out.reshape(B, C, HH, WW).astype(np.float32)
